# revision 1
# baseline (speedup 1.0000x reference)
"""Multi-head attention (B=4, S=2048, M=1024, H=16, D=64) on 8 trn2 cores.

Sharding: core c = (b, g) with b = c // 2 (batch), g = c % 2 (head group of 8
heads).  Each core computes q/k/v projections for its 8 heads, causal
attention, and a partial output projection (contraction over its 512 feature
rows of Wo).  Host sums the two partials per batch and adds the bias.

Device-side layouts (all fp32):
  xqT/xkT/xvT  [1024(m), 2048(s)]   host-transposed activations
  wq/wk/wv     [1024(m), 512(dh)]   dh = 64*h_local + d  (head-major)
  wo           [512(dh), 1024(n)]
  qT,kT        [512(dh), 2048(s)]   = (X W)^T, computed as W.T @ X.T
  v            [2048(s), 520]       per 128-row tile: cols 65h..65h+63 = v_h,
                                    col 65h+64 = 1.0 (softmax denominator)
  scoresT      [sk, sq]             = kT_h.T @ qT_h  (contraction over d=64)
  PT           exp((scoresT + mask)/8)  -- no max subtraction (|scores/8|<~2)
  outT_h       [65, sq]             = [v_h|1].T @ PT ; row 64 = sum_sk PT
  attnT        [512(dh), sq]        = outT_h / denom, heads stacked
  out_partial  [2048(s), 1024(n)]   = attnT.T @ wo   (no bias)
"""

import os
import sys

for _p in ("/opt/trn_rl_repo", "/root/.axon_site/_ro/trn_rl_repo"):
    if os.path.isdir(_p) and _p not in sys.path:
        sys.path.append(_p)

import numpy as np

B, S, M, H, D = 4, 2048, 1024, 16, 64
G = 2               # head groups (cores per batch)
HPG = H // G        # heads per group = 8
DH = HPG * D        # feature rows per group = 512
NCORES = B * G
SB = 512            # sq block (matmul N)
CK = 128            # sk chunk (matmul M / partition)
NJ = S // SB        # 4 sq blocks
NC = S // CK        # 16 sk chunks
MK = M // 128       # 8 m chunks

_PROG_CACHE = {}


def _build_program(variant):
    """variant: 'causal' | 'allones' | 'general'"""
    import concourse.bass as bass
    import concourse.bacc as bacc
    import concourse.mybir as mybir
    from concourse import tile
    from contextlib import ExitStack

    f32 = mybir.dt.float32
    f32r = mybir.dt.float32r
    nc = bacc.Bacc("TRN2", target_bir_lowering=False, debug=False, num_devices=NCORES)

    xqT = nc.dram_tensor("xqT", [M, S], f32, kind="ExternalInput").ap()
    xkT = nc.dram_tensor("xkT", [M, S], f32, kind="ExternalInput").ap()
    xvT = nc.dram_tensor("xvT", [M, S], f32, kind="ExternalInput").ap()
    wq = nc.dram_tensor("wq", [M, DH], f32, kind="ExternalInput").ap()
    wk = nc.dram_tensor("wk", [M, DH], f32, kind="ExternalInput").ap()
    wv = nc.dram_tensor("wv", [M, DH], f32, kind="ExternalInput").ap()
    wo = nc.dram_tensor("wo", [DH, M], f32, kind="ExternalInput").ap()
    tri = nc.dram_tensor("tri", [128, 128], f32, kind="ExternalInput").ap()
    ind8 = nc.dram_tensor("ind8", [8, SB], f32, kind="ExternalInput").ap()
    if variant == "general":
        maskT = nc.dram_tensor("maskT", [S, S], f32, kind="ExternalInput").ap()
    out = nc.dram_tensor("out", [S, M], f32, kind="ExternalOutput").ap()

    with tile.TileContext(nc) as tc, ExitStack() as ctx:
        ep = ctx.enter_context
        ctx.enter_context(nc.allow_low_precision(reason="f32r matmul inputs"))
        dma = nc.sync.dma_start

        w_pool = ep(tc.tile_pool(name="w", bufs=9))
        x_pool = ep(tc.tile_pool(name="x", bufs=10))
        wo_pool = ep(tc.tile_pool(name="wo", bufs=1))
        qT_pool = ep(tc.tile_pool(name="qT", bufs=1))
        kT_pool = ep(tc.tile_pool(name="kT", bufs=1))
        v_pool = ep(tc.tile_pool(name="v", bufs=1))
        pt_pool = ep(tc.tile_pool(name="pt", bufs=6))
        at_pool = ep(tc.tile_pool(name="at", bufs=6))
        nrm_pool = ep(tc.tile_pool(name="nrm", bufs=3))
        out_pool = ep(tc.tile_pool(name="outp", bufs=3))
        misc_pool = ep(tc.tile_pool(name="misc", bufs=1))
        if variant == "general":
            mk_pool = ep(tc.tile_pool(name="mk", bufs=4))

        ps_mm = ep(tc.tile_pool(name="ps_mm", bufs=2, space="PSUM"))
        ps_sc = ep(tc.tile_pool(name="ps_sc", bufs=4, space="PSUM"))
        ps_pv = ep(tc.tile_pool(name="ps_pv", bufs=2, space="PSUM"))

        # constants
        tri_sb = misc_pool.tile([128, 128], f32, name="tri_sb")
        dma(tri_sb[:], tri[:])
        ind8_sb = misc_pool.tile([8, SB], f32r, name="ind8_sb")
        dma(ind8_sb[:], ind8[:].bitcast(f32r))

        qT_sb = [qT_pool.tile([128, S], f32r, name=f"qT{d}") for d in range(4)]
        kT_sb = [kT_pool.tile([128, S], f32r, name=f"kT{d}") for d in range(4)]
        v_sb = [v_pool.tile([128, HPG * 65], f32r, name=f"v{t}") for t in range(NC)]

        for t in range(NC):
            v3 = v_sb[t].bitcast(f32).rearrange("p (h c) -> p h c", h=HPG, c=65)
            nc.gpsimd.memset(v3[:, :, 64:65], 1.0)

        # ---- phase 1: projections, per s-block ----
        for j in range(NJ):
            for w_dram, x_dram, kind in (
                    (wq, xqT, "q"), (wk, xkT, "k"), (wv, xvT, "v")):
                w_ch = []
                for mc in range(MK):
                    wt = w_pool.tile([128, DH], f32r, name=f"w_{kind}{j}_{mc}", tag="w")
                    nc.gpsimd.dma_start(wt[:], w_dram[mc * 128:(mc + 1) * 128, :].bitcast(f32r))
                    w_ch.append(wt)
                x_ch = []
                for mc in range(MK):
                    xt = x_pool.tile([128, SB], f32r, name=f"x_{kind}{j}_{mc}", tag="x")
                    dma(xt[:], x_dram[mc * 128:(mc + 1) * 128, j * SB:(j + 1) * SB].bitcast(f32r))
                    x_ch.append(xt)
                if kind in ("q", "k"):
                    dst = qT_sb if kind == "q" else kT_sb
                    for d in range(4):
                        ps = ps_mm.tile([128, SB], f32, name=f"ps_{kind}{j}_{d}", tag="mm")
                        for mc in range(MK):
                            nc.tensor.matmul(
                                ps[:], w_ch[mc][:, d * 128:(d + 1) * 128], x_ch[mc][:],
                                start=(mc == 0), stop=(mc == MK - 1))
                        nc.vector.tensor_copy(dst[d][:, j * SB:(j + 1) * SB], ps[:])
                else:
                    for st in range(4):
                        t = 4 * j + st
                        ps = ps_mm.tile([128, DH], f32, name=f"ps_v{t}", tag="mm")
                        for mc in range(MK):
                            nc.tensor.matmul(
                                ps[:], x_ch[mc][:, st * 128:(st + 1) * 128], w_ch[mc][:],
                                start=(mc == 0), stop=(mc == MK - 1))
                        v3 = v_sb[t].rearrange("p (h c) -> p h c", h=HPG, c=65)
                        p3 = ps.rearrange("p (h c) -> p h c", h=HPG, c=64)
                        nc.vector.tensor_copy(v3[:, :, 0:64], p3[:])

        wo_sb = []
        for d in range(4):
            wt = wo_pool.tile([128, M], f32r, name=f"wo{d}")
            nc.gpsimd.dma_start(wt[:], wo[d * 128:(d + 1) * 128, :].bitcast(f32r))
            wo_sb.append(wt)

        # ---- phase 2: attention + output projection, per sq block ----
        def emit_score_chunk(j, h, c):
            """scoresT chunk -> exp -> PT tile; returns (pt, o)."""
            dtile, drow = h // 2, 64 * (h % 2)
            sc = ps_sc.tile([128, SB], f32, name=f"sc{j}_{h}_{c}", tag="sc")
            nc.tensor.matmul(
                sc[:],
                kT_sb[dtile][drow:drow + 64, c * CK:(c + 1) * CK],
                qT_sb[dtile][drow:drow + 64, j * SB:(j + 1) * SB],
                start=True, stop=True)
            pt = pt_pool.tile([128, SB], f32r, name=f"pt{j}_{h}_{c}", tag="pt")
            o = 0
            if variant == "causal" and c >= 4 * j:
                o = 128 * (c - 4 * j)
                nc.vector.tensor_add(
                    sc[:, o:o + 128], sc[:, o:o + 128], tri_sb[:])
            elif variant == "general":
                mk = mk_pool.tile([128, SB], f32, name=f"mk{j}_{h}_{c}", tag="mk")
                nc.gpsimd.dma_start(
                    mk[:], maskT[c * CK:(c + 1) * CK, j * SB:(j + 1) * SB])
                nc.vector.tensor_add(sc[:], sc[:], mk[:])
            nc.scalar.activation(
                pt[:, o:SB], sc[:, o:SB],
                mybir.ActivationFunctionType.Exp, scale=0.125)
            return pt, o

        for j in range(NJ):
            nchunks = 4 * (j + 1) if variant == "causal" else NC
            at_tiles = []
            dn_all = nrm_pool.tile([8, SB], f32r, name=f"dn{j}", tag="dn")
            for hp in range(HPG // 2):
                hA, hB = 2 * hp, 2 * hp + 1
                at = at_pool.tile([128, SB], f32r, name=f"at{j}_{hp}", tag="at")
                at_tiles.append(at)
                pvA = ps_pv.tile([65, SB], f32, name=f"pv{j}_{hA}", tag="pv")
                pvB = ps_pv.tile([65, SB], f32, name=f"pv{j}_{hB}", tag="pv")
                for c in range(nchunks):
                    ptA, oA = emit_score_chunk(j, hA, c)
                    ptB, oB = emit_score_chunk(j, hB, c)
                    nc.tensor.matmul(
                        pvA[:, oA:SB], v_sb[c][:, 65 * hA:65 * hA + 65], ptA[:, oA:SB],
                        start=(c == 0), stop=(c == nchunks - 1))
                    nc.tensor.matmul(
                        pvB[:, oB:SB], v_sb[c][:, 65 * hB:65 * hB + 65], ptB[:, oB:SB],
                        start=(c == 0), stop=(c == nchunks - 1))
                for h, pv in ((hA, pvA), (hB, pvB)):
                    drow = 64 * (h % 2)
                    nc.vector.tensor_copy(at[drow:drow + 64, :], pv[0:64, :])
                    dnt = nrm_pool.tile([1, SB], f32r, name=f"dnt{j}_{h}", tag="dnt")
                    nc.vector.tensor_copy(dnt[:], pv[64:65, :])
                    nc.gpsimd.dma_start(dn_all[h:h + 1, :], dnt[:])
            dnr = nrm_pool.tile([8, SB], f32r, name=f"dnr{j}", tag="dnr")
            nc.vector.reciprocal(dnr[:], dn_all[:])
            for hp in range(HPG // 2):
                rb = ps_mm.tile([128, SB], f32, name=f"rb{j}_{hp}", tag="mm")
                nc.tensor.matmul(
                    rb[:], ind8_sb[:, hp * 128:(hp + 1) * 128], dnr[:],
                    start=True, stop=True)
                rb_sb = nrm_pool.tile([128, SB], f32r, name=f"rbs{j}_{hp}", tag="rbs")
                nc.vector.tensor_copy(rb_sb[:], rb[:])
                nc.vector.tensor_mul(at_tiles[hp][:], at_tiles[hp][:], rb_sb[:])

            for ss in range(4):
                for nh in range(2):
                    ps = ps_mm.tile([128, SB], f32, name=f"po{j}_{ss}_{nh}", tag="mm")
                    for d in range(4):
                        nc.tensor.matmul(
                            ps[:],
                            at_tiles[d][:, ss * 128:(ss + 1) * 128],
                            wo_sb[d][:, nh * SB:(nh + 1) * SB],
                            start=(d == 0), stop=(d == 3))
                    ot = out_pool.tile([128, SB], f32, name=f"ot{j}_{ss}_{nh}", tag="ot")
                    nc.vector.tensor_copy(ot[:], ps[:])
                    r0 = j * SB + ss * 128
                    nc.gpsimd.dma_start(
                        out[r0:r0 + 128, nh * SB:(nh + 1) * SB], ot[:])

    nc.compile()
    return nc


def _get_program(variant):
    if variant not in _PROG_CACHE:
        _PROG_CACHE[variant] = _build_program(variant)
    return _PROG_CACHE[variant]


def _host_prep(queries, keys, values, masks, Wq, Wk, Wv):
    """Build the 8 per-core input maps."""
    tril = np.tril(np.ones((S, S), dtype=bool))
    if all(np.array_equal(masks[b], tril) for b in range(B)):
        variant = "causal"
    elif masks.all():
        variant = "allones"
    else:
        variant = "general"

    sq = np.arange(128)
    tri_np = np.where(sq[None, :] >= sq[:, None], 0.0, -1.0e6).astype(np.float32)
    ind8_np = np.zeros((8, 512), np.float32)
    for c in range(4):
        for cc in range(128):
            ind8_np[2 * c + cc // 64, 128 * c + cc] = 1.0

    # [H, M, D] -> [M, H*D] head-major per group
    def wcat(w, g):
        return np.ascontiguousarray(
            w[g * HPG:(g + 1) * HPG].transpose(1, 0, 2).reshape(M, DH))

    in_maps = []
    for c in range(NCORES):
        b, g = c // G, c % G
        m = {
            "xqT": np.ascontiguousarray(queries[b].T),
            "xkT": np.ascontiguousarray(keys[b].T),
            "xvT": np.ascontiguousarray(values[b].T),
            "wq": wcat(Wq, g),
            "wk": wcat(Wk, g),
            "wv": wcat(Wv, g),
            "tri": tri_np,
            "ind8": ind8_np,
        }
        if variant == "general":
            m["maskT"] = np.where(masks[b].T, 0.0, -1.0e6).astype(np.float32)
        in_maps.append(m)
    return variant, in_maps


def run(queries, keys, values, masks, Wq, Wk, Wv, Wo, bo, trace=False):
    from concourse import bass_utils

    queries = np.asarray(queries, np.float32)
    keys = np.asarray(keys, np.float32)
    values = np.asarray(values, np.float32)
    masks = np.asarray(masks, bool)
    Wq = np.asarray(Wq, np.float32)
    Wk = np.asarray(Wk, np.float32)
    Wv = np.asarray(Wv, np.float32)
    Wo = np.asarray(Wo, np.float32)
    bo = np.asarray(bo, np.float32)

    variant, in_maps = _host_prep(queries, keys, values, masks, Wq, Wk, Wv)
    for c in range(NCORES):
        g = c % G
        in_maps[c]["wo"] = np.ascontiguousarray(Wo[g * DH:(g + 1) * DH, :])

    nc = _get_program(variant)
    res = bass_utils.run_bass_kernel_spmd(
        nc, in_maps, list(range(NCORES)), trace=trace)

    out = np.empty((B, S, M), np.float32)
    for b in range(B):
        out[b] = res.results[G * b]["out"] + res.results[G * b + 1]["out"] + bo
    return out, res


def kernel(queries, keys, values, masks, Wq, Wk, Wv, Wo, bo):
    out, _ = run(queries, keys, values, masks, Wq, Wk, Wv, Wo, bo, trace=False)
    return out



# revision 3
# speedup vs baseline: 1.3116x; 1.3116x over previous
"""Multi-head attention (B=4, S=2048, M=1024, H=16, D=64) on 8 trn2 cores.

Sharding: core c = (b, g) with b = c // 2 (batch), g = c % 2 (head group of 8
heads).  Each core computes q/k/v projections for its 8 heads, causal
attention, and a partial output projection (contraction over its 512 feature
rows of Wo).  Host sums the two partials per batch and adds the bias.

All matmul operands are fp16 (fp32 matmul trips the PE power throttle to a
0.5 util limit on trn2; fp16 streams 1 row/cycle at full 2.4 GHz).  PSUM
accumulation stays fp32.  Weights stay SBUF-resident; x activations stream
per 512-column s-block; projections for block j are emitted immediately
before attention for block j so the Act-engine exp work overlaps the PE.

Device-side layouts:
  xqT/xkT/xvT  [1024(m), 2048(s)]  fp16 host-transposed activations
  wq/wk/wv     [1024(m), 512(dh)]  fp16, dh = 64*h_local + d  (head-major)
  wo           [512(dh), 1024(n)]  fp16
  qT,kT        [512(dh), 2048(s)]  fp16 = (X W)^T, computed as W.T @ X.T
  v            [2048(s), 520]      fp16; per 128-row tile: cols 65h..65h+63
                                   = v_h, col 65h+64 = 1.0 (softmax denom)
  scoresT      [sk, sq]            = kT_h.T @ qT_h  (contraction over d=64)
  PT           exp((scoresT + mask)/8) fp16 -- no max subtraction
  outT_h       [65, sq]            = [v_h|1].T @ PT ; row 64 = sum_sk PT
  attnT        [512(dh), sq]       = outT_h / denom, heads stacked, fp16
  out_partial  [2048(s), 1024(n)]  fp16 = attnT.T @ wo   (no bias)
"""

import os
import sys

for _p in ("/opt/trn_rl_repo", "/root/.axon_site/_ro/trn_rl_repo"):
    if os.path.isdir(_p) and _p not in sys.path:
        sys.path.append(_p)

import numpy as np

B, S, M, H, D = 4, 2048, 1024, 16, 64
G = 2               # head groups (cores per batch)
HPG = H // G        # heads per group = 8
DH = HPG * D        # feature rows per group = 512
NCORES = B * G
SB = 512            # sq block (matmul N)
CK = 128            # sk chunk (matmul M / partition)
NJ = S // SB        # 4 sq blocks
NC = S // CK        # 16 sk chunks
MK = M // 128       # 8 m chunks

_PROG_CACHE = {}


def _build_program(variant):
    """variant: 'causal' | 'allones' | 'general'"""
    import concourse.bass as bass
    import concourse.bacc as bacc
    import concourse.mybir as mybir
    from concourse import tile
    from contextlib import ExitStack

    f32 = mybir.dt.float32
    f16 = mybir.dt.float16
    nc = bacc.Bacc("TRN2", target_bir_lowering=False, debug=False, num_devices=NCORES)

    xqT = nc.dram_tensor("xqT", [M, S], f16, kind="ExternalInput").ap()
    xkT = nc.dram_tensor("xkT", [M, S], f16, kind="ExternalInput").ap()
    xvT = nc.dram_tensor("xvT", [M, S], f16, kind="ExternalInput").ap()
    wq = nc.dram_tensor("wq", [M, DH], f16, kind="ExternalInput").ap()
    wk = nc.dram_tensor("wk", [M, DH], f16, kind="ExternalInput").ap()
    wv = nc.dram_tensor("wv", [M, DH], f16, kind="ExternalInput").ap()
    wo = nc.dram_tensor("wo", [DH, M], f16, kind="ExternalInput").ap()
    tri = nc.dram_tensor("tri", [128, 128], f32, kind="ExternalInput").ap()
    ind8 = nc.dram_tensor("ind8", [8, SB], f16, kind="ExternalInput").ap()
    if variant == "general":
        maskT = nc.dram_tensor("maskT", [S, S], f32, kind="ExternalInput").ap()
    out = nc.dram_tensor("out", [S, M], f16, kind="ExternalOutput").ap()

    with tile.TileContext(nc) as tc, ExitStack() as ctx:
        ep = ctx.enter_context
        ctx.enter_context(nc.allow_low_precision(reason="fp16 matmul inputs"))
        dma = nc.sync.dma_start

        w_pool = ep(tc.tile_pool(name="w", bufs=24))
        x_pool = ep(tc.tile_pool(name="x", bufs=12))
        wo_pool = ep(tc.tile_pool(name="wo", bufs=1))
        qT_pool = ep(tc.tile_pool(name="qT", bufs=1))
        kT_pool = ep(tc.tile_pool(name="kT", bufs=1))
        v_pool = ep(tc.tile_pool(name="v", bufs=1))
        pt_pool = ep(tc.tile_pool(name="pt", bufs=6))
        at_pool = ep(tc.tile_pool(name="at", bufs=6))
        nrm_pool = ep(tc.tile_pool(name="nrm", bufs=3))
        out_pool = ep(tc.tile_pool(name="outp", bufs=3))
        misc_pool = ep(tc.tile_pool(name="misc", bufs=1))
        if variant == "general":
            mk_pool = ep(tc.tile_pool(name="mk", bufs=4))

        ps_mm = ep(tc.tile_pool(name="ps_mm", bufs=2, space="PSUM"))
        ps_sc = ep(tc.tile_pool(name="ps_sc", bufs=4, space="PSUM"))
        ps_pv = ep(tc.tile_pool(name="ps_pv", bufs=2, space="PSUM"))

        # constants
        tri_sb = misc_pool.tile([128, 128], f32, name="tri_sb")
        dma(tri_sb[:], tri[:])
        ind8_sb = misc_pool.tile([8, SB], f16, name="ind8_sb")
        dma(ind8_sb[:], ind8[:])

        qT_sb = [qT_pool.tile([128, S], f16, name=f"qT{d}") for d in range(4)]
        kT_sb = [kT_pool.tile([128, S], f16, name=f"kT{d}") for d in range(4)]
        v_sb = [v_pool.tile([128, HPG * 65], f16, name=f"v{t}") for t in range(NC)]

        for t in range(NC):
            v3 = v_sb[t].rearrange("p (h c) -> p h c", h=HPG, c=65)
            nc.gpsimd.memset(v3[:, :, 64:65], 1.0)

        # resident weights: loaded once up front (gpsimd queue, idle at start)
        w_sb = {}
        for w_dram, kind in ((wq, "q"), (wk, "k"), (wv, "v")):
            ch = []
            for mc in range(MK):
                wt = w_pool.tile([128, DH], f16, name=f"w_{kind}{mc}", tag="w")
                nc.gpsimd.dma_start(wt[:], w_dram[mc * 128:(mc + 1) * 128, :])
                ch.append(wt)
            w_sb[kind] = ch
        wo_sb = []
        for d in range(4):
            wt = wo_pool.tile([128, M], f16, name=f"wo{d}")
            nc.gpsimd.dma_start(wt[:], wo[d * 128:(d + 1) * 128, :])
            wo_sb.append(wt)

        def emit_proj(j):
            for x_dram, kind in ((xqT, "q"), (xkT, "k"), (xvT, "v")):
                x_ch = []
                for mc in range(MK):
                    xt = x_pool.tile([128, SB], f16, name=f"x_{kind}{j}_{mc}", tag="x")
                    dma(xt[:], x_dram[mc * 128:(mc + 1) * 128, j * SB:(j + 1) * SB])
                    x_ch.append(xt)
                w_ch = w_sb[kind]
                if kind in ("q", "k"):
                    dst = qT_sb if kind == "q" else kT_sb
                    for d in range(4):
                        ps = ps_mm.tile([128, SB], f32, name=f"ps_{kind}{j}_{d}", tag="mm")
                        for mc in range(MK):
                            nc.tensor.matmul(
                                ps[:], w_ch[mc][:, d * 128:(d + 1) * 128], x_ch[mc][:],
                                start=(mc == 0), stop=(mc == MK - 1))
                        nc.vector.tensor_copy(dst[d][:, j * SB:(j + 1) * SB], ps[:])
                else:
                    for st in range(4):
                        t = 4 * j + st
                        ps = ps_mm.tile([128, DH], f32, name=f"ps_v{t}", tag="mm")
                        for mc in range(MK):
                            nc.tensor.matmul(
                                ps[:], x_ch[mc][:, st * 128:(st + 1) * 128], w_ch[mc][:],
                                start=(mc == 0), stop=(mc == MK - 1))
                        v3 = v_sb[t].rearrange("p (h c) -> p h c", h=HPG, c=65)
                        p3 = ps.rearrange("p (h c) -> p h c", h=HPG, c=64)
                        nc.vector.tensor_copy(v3[:, :, 0:64], p3[:])

        def emit_score_chunk(j, h, c):
            """scoresT chunk -> exp -> PT tile; returns (pt, o)."""
            dtile, drow = h // 2, 64 * (h % 2)
            o = 0
            if variant == "causal" and c >= 4 * j:
                o = 128 * (c - 4 * j)
            sc = ps_sc.tile([128, SB], f32, name=f"sc{j}_{h}_{c}", tag="sc")
            nc.tensor.matmul(
                sc[:, o:SB],
                kT_sb[dtile][drow:drow + 64, c * CK:(c + 1) * CK],
                qT_sb[dtile][drow:drow + 64, j * SB + o:(j + 1) * SB],
                start=True, stop=True)
            pt = pt_pool.tile([128, SB], f16, name=f"pt{j}_{h}_{c}", tag="pt")
            if variant == "causal" and c >= 4 * j:
                nc.vector.tensor_add(
                    sc[:, o:o + 128], sc[:, o:o + 128], tri_sb[:])
            elif variant == "general":
                mk = mk_pool.tile([128, SB], f32, name=f"mk{j}_{h}_{c}", tag="mk")
                nc.gpsimd.dma_start(
                    mk[:], maskT[c * CK:(c + 1) * CK, j * SB:(j + 1) * SB])
                nc.vector.tensor_add(sc[:], sc[:], mk[:])
            nc.scalar.activation(
                pt[:, o:SB], sc[:, o:SB],
                mybir.ActivationFunctionType.Exp, scale=0.125)
            return pt, o

        def emit_attn(j):
            nchunks = 4 * (j + 1) if variant == "causal" else NC
            at_tiles = []
            dn_all = nrm_pool.tile([8, SB], f32, name=f"dn{j}", tag="dn")
            for hp in range(HPG // 2):
                hA, hB = 2 * hp, 2 * hp + 1
                at = at_pool.tile([128, SB], f16, name=f"at{j}_{hp}", tag="at")
                at_tiles.append(at)
                pvA = ps_pv.tile([65, SB], f32, name=f"pv{j}_{hA}", tag="pv")
                pvB = ps_pv.tile([65, SB], f32, name=f"pv{j}_{hB}", tag="pv")
                for c in range(nchunks):
                    ptA, oA = emit_score_chunk(j, hA, c)
                    ptB, oB = emit_score_chunk(j, hB, c)
                    nc.tensor.matmul(
                        pvA[:, oA:SB], v_sb[c][:, 65 * hA:65 * hA + 65], ptA[:, oA:SB],
                        start=(c == 0), stop=(c == nchunks - 1))
                    nc.tensor.matmul(
                        pvB[:, oB:SB], v_sb[c][:, 65 * hB:65 * hB + 65], ptB[:, oB:SB],
                        start=(c == 0), stop=(c == nchunks - 1))
                for h, pv in ((hA, pvA), (hB, pvB)):
                    drow = 64 * (h % 2)
                    nc.vector.tensor_copy(at[drow:drow + 64, :], pv[0:64, :])
                    # DVE writes must start at a quarter partition; bounce the
                    # denominator row through partition 0, DMA to row h
                    dnt = nrm_pool.tile([1, SB], f32, name=f"dnt{j}_{h}", tag="dnt")
                    nc.vector.tensor_copy(dnt[:], pv[64:65, :])
                    nc.gpsimd.dma_start(dn_all[h:h + 1, :], dnt[:])
            dnr = nrm_pool.tile([8, SB], f16, name=f"dnr{j}", tag="dnr")
            nc.vector.reciprocal(dnr[:], dn_all[:])
            for hp in range(HPG // 2):
                rb = ps_mm.tile([128, SB], f32, name=f"rb{j}_{hp}", tag="mm")
                nc.tensor.matmul(
                    rb[:], ind8_sb[:, hp * 128:(hp + 1) * 128], dnr[:],
                    start=True, stop=True)
                nc.vector.tensor_mul(at_tiles[hp][:], at_tiles[hp][:], rb[:])

            for ss in range(4):
                for nh in range(2):
                    ps = ps_mm.tile([128, SB], f32, name=f"po{j}_{ss}_{nh}", tag="mm")
                    for d in range(4):
                        nc.tensor.matmul(
                            ps[:],
                            at_tiles[d][:, ss * 128:(ss + 1) * 128],
                            wo_sb[d][:, nh * SB:(nh + 1) * SB],
                            start=(d == 0), stop=(d == 3))
                    ot = out_pool.tile([128, SB], f16, name=f"ot{j}_{ss}_{nh}", tag="ot")
                    nc.vector.tensor_copy(ot[:], ps[:])
                    r0 = j * SB + ss * 128
                    nc.gpsimd.dma_start(
                        out[r0:r0 + 128, nh * SB:(nh + 1) * SB], ot[:])

        # interleave: proj(j) then attn(j); attn(j) only needs k/v blocks <= j
        for j in range(NJ):
            emit_proj(j)
            emit_attn(j)

    nc.compile()
    return nc


def _get_program(variant):
    if variant not in _PROG_CACHE:
        _PROG_CACHE[variant] = _build_program(variant)
    return _PROG_CACHE[variant]


def _host_prep(queries, keys, values, masks, Wq, Wk, Wv):
    """Build the 8 per-core input maps."""
    tril = np.tril(np.ones((S, S), dtype=bool))
    if all(np.array_equal(masks[b], tril) for b in range(B)):
        variant = "causal"
    elif masks.all():
        variant = "allones"
    else:
        variant = "general"

    sq = np.arange(128)
    tri_np = np.where(sq[None, :] >= sq[:, None], 0.0, -1.0e6).astype(np.float32)
    ind8_np = np.zeros((8, 512), np.float16)
    for c in range(4):
        for cc in range(128):
            ind8_np[2 * c + cc // 64, 128 * c + cc] = 1.0

    # [H, M, D] -> [M, H*D] head-major per group
    def wcat(w, g):
        return np.ascontiguousarray(
            w[g * HPG:(g + 1) * HPG].transpose(1, 0, 2).reshape(M, DH)
        ).astype(np.float16)

    in_maps = []
    for c in range(NCORES):
        b, g = c // G, c % G
        m = {
            "xqT": np.ascontiguousarray(queries[b].T).astype(np.float16),
            "xkT": np.ascontiguousarray(keys[b].T).astype(np.float16),
            "xvT": np.ascontiguousarray(values[b].T).astype(np.float16),
            "wq": wcat(Wq, g),
            "wk": wcat(Wk, g),
            "wv": wcat(Wv, g),
            "tri": tri_np,
            "ind8": ind8_np,
        }
        if variant == "general":
            m["maskT"] = np.where(masks[b].T, 0.0, -1.0e6).astype(np.float32)
        in_maps.append(m)
    return variant, in_maps


def run(queries, keys, values, masks, Wq, Wk, Wv, Wo, bo, trace=False):
    from concourse import bass_utils

    queries = np.asarray(queries, np.float32)
    keys = np.asarray(keys, np.float32)
    values = np.asarray(values, np.float32)
    masks = np.asarray(masks, bool)
    Wq = np.asarray(Wq, np.float32)
    Wk = np.asarray(Wk, np.float32)
    Wv = np.asarray(Wv, np.float32)
    Wo = np.asarray(Wo, np.float32)
    bo = np.asarray(bo, np.float32)

    variant, in_maps = _host_prep(queries, keys, values, masks, Wq, Wk, Wv)
    for c in range(NCORES):
        g = c % G
        in_maps[c]["wo"] = np.ascontiguousarray(
            Wo[g * DH:(g + 1) * DH, :]).astype(np.float16)

    nc = _get_program(variant)
    res = bass_utils.run_bass_kernel_spmd(
        nc, in_maps, list(range(NCORES)), trace=trace)

    out = np.empty((B, S, M), np.float32)
    for b in range(B):
        out[b] = (res.results[G * b]["out"].astype(np.float32)
                  + res.results[G * b + 1]["out"].astype(np.float32) + bo)
    return out, res


def kernel(queries, keys, values, masks, Wq, Wk, Wv, Wo, bo):
    out, _ = run(queries, keys, values, masks, Wq, Wk, Wv, Wo, bo, trace=False)
    return out


# revision 5
# speedup vs baseline: 2.0422x; 1.5570x over previous
"""Multi-head attention (B=4, S=2048, M=1024, H=16, D=64) on 8 trn2 cores.

Sharding: core c = (b, g) with b = c // 2 (batch), g = c % 2 (head group of 8
heads).  Each core computes q/k/v projections for its 8 heads, causal
attention, and a partial output projection (contraction over its 512 feature
rows of Wo).  Host sums the two partials per batch and adds the bias.

All matmul operands are fp16 (fp32 matmul trips the PE power throttle to a
0.5 util limit on trn2; fp16 streams 1 row/cycle).  PSUM accumulation stays
fp32.  Weights stay SBUF-resident.

Pipelining: the PE runs in emission order, so the per-block work is emitted
as  proj(0), attn(0){fill: proj(1)}, attn(1){fill: proj(2), finish(0)},
attn(2){fill: proj(3), finish(1)}, attn(3){fill: finish(2)}, finish(3),
where attn(j) is the score/exp/PV pipeline (Act-engine bound) and the
fillers are PE-only closures drained at head-pair boundaries to plug the
exp-gated PE gaps.  finish(j) = softmax normalization (rb broadcast matmul +
multiply) and the output projection for block j; deferring it hides the
denominator-reciprocal latency entirely.

Device-side layouts:
  xqT/xkT/xvT  [1024(m), 2048(s)]  fp16 host-transposed activations
  wq/wk/wv     [1024(m), 512(dh)]  fp16, dh = 64*h_local + d  (head-major)
  wo           [512(dh), 1024(n)]  fp16
  qT,kT        [512(dh), 2048(s)]  fp16 = (X W)^T, computed as W.T @ X.T
  v            [2048(s), 520]      fp16; per 128-row tile: cols 65h..65h+63
                                   = v_h, col 65h+64 = 1.0 (softmax denom)
  scoresT      [sk, sq]            = kT_h.T @ qT_h, head pair packed in one
                                   [128, 1024] PSUM tile (A cols 0:512, B
                                   cols 512:1024) so one exp covers both
  PT           exp(scoresT/8) fp16; causal diag masked AFTER exp by a 0/1
                                   lower-tri multiply (keeps Act off the
                                   DVE's critical path)
  outT_h       [65, sq]            = [v_h|1].T @ PT ; row 64 = sum_sk PT
  attnT        [512(dh), sq]       = outT_h / denom, heads stacked, fp16
  out_partial  [2048(s), 1024(n)]  fp16 = attnT.T @ wo   (no bias)
"""

import os
import sys

for _p in ("/opt/trn_rl_repo", "/root/.axon_site/_ro/trn_rl_repo"):
    if os.path.isdir(_p) and _p not in sys.path:
        sys.path.append(_p)

import numpy as np

B, S, M, H, D = 4, 2048, 1024, 16, 64
G = 2               # head groups (cores per batch)
HPG = H // G        # heads per group = 8
DH = HPG * D        # feature rows per group = 512
NCORES = B * G
SB = 512            # sq block (matmul N)
CK = 128            # sk chunk (matmul M / partition)
NJ = S // SB        # 4 sq blocks
NC = S // CK        # 16 sk chunks
MK = M // 128       # 8 m chunks

_PROG_CACHE = {}


def _build_program(variant):
    """variant: 'causal' | 'allones' | 'general'"""
    import concourse.bass as bass
    import concourse.bacc as bacc
    import concourse.mybir as mybir
    from concourse import tile
    from contextlib import ExitStack

    f32 = mybir.dt.float32
    f16 = mybir.dt.float16
    nc = bacc.Bacc("TRN2", target_bir_lowering=False, debug=False, num_devices=NCORES)

    xqT = nc.dram_tensor("xqT", [M, S], f16, kind="ExternalInput").ap()
    xkT = nc.dram_tensor("xkT", [M, S], f16, kind="ExternalInput").ap()
    xvT = nc.dram_tensor("xvT", [M, S], f16, kind="ExternalInput").ap()
    wq = nc.dram_tensor("wq", [M, DH], f16, kind="ExternalInput").ap()
    wk = nc.dram_tensor("wk", [M, DH], f16, kind="ExternalInput").ap()
    wv = nc.dram_tensor("wv", [M, DH], f16, kind="ExternalInput").ap()
    wo = nc.dram_tensor("wo", [DH, M], f16, kind="ExternalInput").ap()
    ind8 = nc.dram_tensor("ind8", [8, SB], f16, kind="ExternalInput").ap()
    if variant == "causal":
        tri01 = nc.dram_tensor("tri01", [128, 128], f16, kind="ExternalInput").ap()
    if variant == "general":
        maskT = nc.dram_tensor("maskT", [S, S], f32, kind="ExternalInput").ap()
    out = nc.dram_tensor("out", [S, M], f16, kind="ExternalOutput").ap()

    with tile.TileContext(nc) as tc, ExitStack() as ctx:
        ep = ctx.enter_context
        ctx.enter_context(nc.allow_low_precision(reason="fp16 matmul inputs"))
        dma = nc.sync.dma_start

        w_pool = ep(tc.tile_pool(name="w", bufs=24))
        x_pool = ep(tc.tile_pool(name="x", bufs=20))
        wo_pool = ep(tc.tile_pool(name="wo", bufs=1))
        qT_pool = ep(tc.tile_pool(name="qT", bufs=1))
        kT_pool = ep(tc.tile_pool(name="kT", bufs=1))
        v_pool = ep(tc.tile_pool(name="v", bufs=1))
        pt_pool = ep(tc.tile_pool(name="pt", bufs=4))
        at_pool = ep(tc.tile_pool(name="at", bufs=9))
        nrm_pool = ep(tc.tile_pool(name="nrm", bufs=3))
        out_pool = ep(tc.tile_pool(name="outp", bufs=4))
        misc_pool = ep(tc.tile_pool(name="misc", bufs=1))
        if variant == "general":
            mk_pool = ep(tc.tile_pool(name="mk", bufs=4))

        ps_mm = ep(tc.tile_pool(name="ps_mm", bufs=2, space="PSUM"))
        ps_sc = ep(tc.tile_pool(name="ps_sc", bufs=2, space="PSUM"))
        ps_pv = ep(tc.tile_pool(name="ps_pv", bufs=2, space="PSUM"))

        # constants
        if variant == "causal":
            tri_sb = misc_pool.tile([128, 128], f16, name="tri_sb")
            dma(tri_sb[:], tri01[:])
        ind8_sb = misc_pool.tile([8, SB], f16, name="ind8_sb")
        dma(ind8_sb[:], ind8[:])

        qT_sb = [qT_pool.tile([128, S], f16, name=f"qT{d}") for d in range(4)]
        kT_sb = [kT_pool.tile([128, S], f16, name=f"kT{d}") for d in range(4)]
        v_sb = [v_pool.tile([128, HPG * 65], f16, name=f"v{t}") for t in range(NC)]

        for t in range(NC):
            v3 = v_sb[t].rearrange("p (h c) -> p h c", h=HPG, c=65)
            nc.gpsimd.memset(v3[:, :, 64:65], 1.0)

        # resident weights: loaded once up front (gpsimd queue, idle at start)
        w_sb = {}
        for w_dram, kind in ((wq, "q"), (wk, "k"), (wv, "v")):
            ch = []
            for mc in range(MK):
                wt = w_pool.tile([128, DH], f16, name=f"w_{kind}{mc}", tag="w")
                nc.gpsimd.dma_start(wt[:], w_dram[mc * 128:(mc + 1) * 128, :])
                ch.append(wt)
            w_sb[kind] = ch
        wo_sb = []
        for d in range(4):
            wt = wo_pool.tile([128, M], f16, name=f"wo{d}")
            nc.gpsimd.dma_start(wt[:], wo[d * 128:(d + 1) * 128, :])
            wo_sb.append(wt)

        def proj_closures(j):
            """12 PE-group closures computing qT/kT/v for s-block j."""
            closures = []
            for x_dram, kind in ((xqT, "q"), (xkT, "k"), (xvT, "v")):
                x_ch = [None] * MK

                def load_x(kind=kind, x_dram=x_dram, x_ch=x_ch):
                    for mc in range(MK):
                        xt = x_pool.tile(
                            [128, SB], f16, name=f"x_{kind}{j}_{mc}", tag="x")
                        dma(xt[:], x_dram[mc * 128:(mc + 1) * 128,
                                          j * SB:(j + 1) * SB])
                        x_ch[mc] = xt

                if kind in ("q", "k"):
                    dst = qT_sb if kind == "q" else kT_sb

                    def group(d, kind=kind, x_ch=x_ch, dst=dst, load_x=load_x):
                        if d == 0:
                            load_x()
                        ps = ps_mm.tile(
                            [128, SB], f32, name=f"ps_{kind}{j}_{d}", tag="mm")
                        for mc in range(MK):
                            nc.tensor.matmul(
                                ps[:], w_sb[kind][mc][:, d * 128:(d + 1) * 128],
                                x_ch[mc][:],
                                start=(mc == 0), stop=(mc == MK - 1))
                        nc.vector.tensor_copy(
                            dst[d][:, j * SB:(j + 1) * SB], ps[:])
                else:
                    def group(st, x_ch=x_ch, load_x=load_x):
                        if st == 0:
                            load_x()
                        t = 4 * j + st
                        ps = ps_mm.tile([128, DH], f32, name=f"ps_v{t}", tag="mm")
                        for mc in range(MK):
                            nc.tensor.matmul(
                                ps[:], x_ch[mc][:, st * 128:(st + 1) * 128],
                                w_sb["v"][mc][:],
                                start=(mc == 0), stop=(mc == MK - 1))
                        v3 = v_sb[t].rearrange("p (h c) -> p h c", h=HPG, c=65)
                        p3 = ps.rearrange("p (h c) -> p h c", h=HPG, c=64)
                        nc.vector.tensor_copy(v3[:, :, 0:64], p3[:])

                for d in range(4):
                    closures.append(lambda d=d, group=group: group(d))
            return closures

        def emit_attn(j, fillers):
            """Scores / exp / PV for s-block j; drains filler closures at
            head-pair boundaries (and mid-chunk-loop for the big blocks).
            Returns the state needed by finish_closures."""
            nchunks = 4 * (j + 1) if variant == "causal" else NC
            fq = list(fillers)
            nhp = HPG // 2
            # distribute fillers over (hp, drain-site) slots
            drains_per_hp = max(1, (len(fq) + nhp - 1) // nhp) if fq else 0

            def drain(n):
                for _ in range(n):
                    if fq:
                        fq.pop(0)()

            at_tiles = []
            dn_all = nrm_pool.tile([8, SB], f32, name=f"dn{j}", tag="dn")
            for hp in range(nhp):
                hA, hB = 2 * hp, 2 * hp + 1
                dtile = hp  # == hA//2 == hB//2
                at = at_pool.tile([128, SB], f16, name=f"at{j}_{hp}", tag="at")
                at_tiles.append(at)
                pvA = ps_pv.tile([65, SB], f32, name=f"pv{j}_{hA}", tag="pv")
                pvB = ps_pv.tile([65, SB], f32, name=f"pv{j}_{hB}", tag="pv")
                for c in range(nchunks):
                    o = 0
                    diag = variant == "causal" and c >= 4 * j
                    if diag:
                        o = 128 * (c - 4 * j)
                    # paired scores: head A cols [o:SB], head B [SB+o:2*SB]
                    sc = ps_sc.tile([128, 2 * SB], f32,
                                    name=f"sc{j}_{hp}_{c}", tag="sc")
                    nc.tensor.matmul(
                        sc[:, o:SB],
                        kT_sb[dtile][0:64, c * CK:(c + 1) * CK],
                        qT_sb[dtile][0:64, j * SB + o:(j + 1) * SB],
                        start=True, stop=True)
                    nc.tensor.matmul(
                        sc[:, SB + o:2 * SB],
                        kT_sb[dtile][64:128, c * CK:(c + 1) * CK],
                        qT_sb[dtile][64:128, j * SB + o:(j + 1) * SB],
                        start=True, stop=True)
                    pt = pt_pool.tile([128, 2 * SB], f16,
                                      name=f"pt{j}_{hp}_{c}", tag="pt")
                    if variant == "general":
                        mk = mk_pool.tile([128, SB], f32,
                                          name=f"mk{j}_{hp}_{c}", tag="mk")
                        nc.gpsimd.dma_start(
                            mk[:], maskT[c * CK:(c + 1) * CK,
                                         j * SB:(j + 1) * SB])
                        nc.vector.tensor_add(sc[:, 0:SB], sc[:, 0:SB], mk[:])
                        nc.vector.tensor_add(
                            sc[:, SB:2 * SB], sc[:, SB:2 * SB], mk[:])
                    if diag:
                        nc.scalar.activation(
                            pt[:, o:SB], sc[:, o:SB],
                            mybir.ActivationFunctionType.Exp, scale=0.125)
                        nc.scalar.activation(
                            pt[:, SB + o:2 * SB], sc[:, SB + o:2 * SB],
                            mybir.ActivationFunctionType.Exp, scale=0.125)
                        # mask after exp: multiply diag 128-col block by 0/1 tri
                        nc.vector.tensor_mul(
                            pt[:, o:o + 128], pt[:, o:o + 128], tri_sb[:])
                        nc.vector.tensor_mul(
                            pt[:, SB + o:SB + o + 128],
                            pt[:, SB + o:SB + o + 128], tri_sb[:])
                    else:
                        nc.scalar.activation(
                            pt[:, 0:2 * SB], sc[:, 0:2 * SB],
                            mybir.ActivationFunctionType.Exp, scale=0.125)
                    nc.tensor.matmul(
                        pvA[:, o:SB], v_sb[c][:, 65 * hA:65 * hA + 65],
                        pt[:, o:SB],
                        start=(c == 0), stop=(c == nchunks - 1))
                    nc.tensor.matmul(
                        pvB[:, o:SB], v_sb[c][:, 65 * hB:65 * hB + 65],
                        pt[:, SB + o:2 * SB],
                        start=(c == 0), stop=(c == nchunks - 1))
                    # mid-loop drip for the long chunk loops
                    if nchunks >= 12 and c in (5, 10):
                        drain(1)
                for h, pv in ((hA, pvA), (hB, pvB)):
                    drow = 64 * (h % 2)
                    nc.vector.tensor_copy(at[drow:drow + 64, :], pv[0:64, :])
                    # DVE writes must start at a quarter partition; bounce the
                    # denominator row through partition 0, DMA to row h
                    dnt = nrm_pool.tile([1, SB], f32, name=f"dnt{j}_{h}",
                                        tag="dnt")
                    nc.vector.tensor_copy(dnt[:], pv[64:65, :])
                    nc.gpsimd.dma_start(dn_all[h:h + 1, :], dnt[:])
                drain(drains_per_hp)
            drain(len(fq))
            # approx reciprocal (~5x cheaper than nc.vector.reciprocal; the
            # denominators are O(1e2..1e3), far from its undefined edges)
            dnr32 = nrm_pool.tile([8, SB], f32, name=f"dnr32{j}", tag="dnr32")
            nc.vector.reciprocal_approx_fast(dnr32[:], dn_all[:])
            dnr = nrm_pool.tile([8, SB], f16, name=f"dnr{j}", tag="dnr")
            nc.vector.tensor_copy(dnr[:], dnr32[:])
            return at_tiles, dnr

        def finish_closures(j, at_tiles, dnr):
            """12 PE closures: normalize attnT and run the output projection."""
            closures = []

            def rb_mul(hp):
                rb = ps_mm.tile([128, SB], f32, name=f"rb{j}_{hp}", tag="mm")
                nc.tensor.matmul(
                    rb[:], ind8_sb[:, hp * 128:(hp + 1) * 128], dnr[:],
                    start=True, stop=True)
                nc.vector.tensor_mul(at_tiles[hp][:], at_tiles[hp][:], rb[:])

            def outproj(ss, nh):
                ps = ps_mm.tile([128, SB], f32, name=f"po{j}_{ss}_{nh}", tag="mm")
                for d in range(4):
                    nc.tensor.matmul(
                        ps[:],
                        at_tiles[d][:, ss * 128:(ss + 1) * 128],
                        wo_sb[d][:, nh * SB:(nh + 1) * SB],
                        start=(d == 0), stop=(d == 3))
                ot = out_pool.tile([128, SB], f16, name=f"ot{j}_{ss}_{nh}",
                                   tag="ot")
                nc.vector.tensor_copy(ot[:], ps[:])
                r0 = j * SB + ss * 128
                nc.gpsimd.dma_start(
                    out[r0:r0 + 128, nh * SB:(nh + 1) * SB], ot[:])

            for hp in range(HPG // 2):
                closures.append(lambda hp=hp: rb_mul(hp))
            for ss in range(4):
                for nh in range(2):
                    closures.append(lambda ss=ss, nh=nh: outproj(ss, nh))
            return closures

        # proj(0) runs alone; attn(j) drains proj(j+1) + finish(j-1) fillers
        for cl in proj_closures(0):
            cl()
        pending = []
        for j in range(NJ):
            fillers = (proj_closures(j + 1) if j + 1 < NJ else []) + pending
            at_tiles, dnr = emit_attn(j, fillers)
            pending = finish_closures(j, at_tiles, dnr)
        for cl in pending:
            cl()

    nc.compile()
    return nc


def _get_program(variant):
    if variant not in _PROG_CACHE:
        _PROG_CACHE[variant] = _build_program(variant)
    return _PROG_CACHE[variant]


def _host_prep(queries, keys, values, masks, Wq, Wk, Wv):
    """Build the 8 per-core input maps."""
    tril = np.tril(np.ones((S, S), dtype=bool))
    if all(np.array_equal(masks[b], tril) for b in range(B)):
        variant = "causal"
    elif masks.all():
        variant = "allones"
    else:
        variant = "general"

    sq = np.arange(128)
    tri01_np = (sq[None, :] >= sq[:, None]).astype(np.float16)
    ind8_np = np.zeros((8, 512), np.float16)
    for c in range(4):
        for cc in range(128):
            ind8_np[2 * c + cc // 64, 128 * c + cc] = 1.0

    # [H, M, D] -> [M, H*D] head-major per group
    def wcat(w, g):
        return np.ascontiguousarray(
            w[g * HPG:(g + 1) * HPG].transpose(1, 0, 2).reshape(M, DH)
        ).astype(np.float16)

    in_maps = []
    for c in range(NCORES):
        b, g = c // G, c % G
        m = {
            "xqT": np.ascontiguousarray(queries[b].T).astype(np.float16),
            "xkT": np.ascontiguousarray(keys[b].T).astype(np.float16),
            "xvT": np.ascontiguousarray(values[b].T).astype(np.float16),
            "wq": wcat(Wq, g),
            "wk": wcat(Wk, g),
            "wv": wcat(Wv, g),
            "ind8": ind8_np,
        }
        if variant == "causal":
            m["tri01"] = tri01_np
        if variant == "general":
            m["maskT"] = np.where(masks[b].T, 0.0, -1.0e6).astype(np.float32)
        in_maps.append(m)
    return variant, in_maps


def run(queries, keys, values, masks, Wq, Wk, Wv, Wo, bo, trace=False):
    from concourse import bass_utils

    queries = np.asarray(queries, np.float32)
    keys = np.asarray(keys, np.float32)
    values = np.asarray(values, np.float32)
    masks = np.asarray(masks, bool)
    Wq = np.asarray(Wq, np.float32)
    Wk = np.asarray(Wk, np.float32)
    Wv = np.asarray(Wv, np.float32)
    Wo = np.asarray(Wo, np.float32)
    bo = np.asarray(bo, np.float32)

    variant, in_maps = _host_prep(queries, keys, values, masks, Wq, Wk, Wv)
    for c in range(NCORES):
        g = c % G
        in_maps[c]["wo"] = np.ascontiguousarray(
            Wo[g * DH:(g + 1) * DH, :]).astype(np.float16)

    nc = _get_program(variant)
    res = bass_utils.run_bass_kernel_spmd(
        nc, in_maps, list(range(NCORES)), trace=trace)

    out = np.empty((B, S, M), np.float32)
    for b in range(B):
        out[b] = (res.results[G * b]["out"].astype(np.float32)
                  + res.results[G * b + 1]["out"].astype(np.float32) + bo)
    return out, res


def kernel(queries, keys, values, masks, Wq, Wk, Wv, Wo, bo):
    out, _ = run(queries, keys, values, masks, Wq, Wk, Wv, Wo, bo, trace=False)
    return out


# revision 13
# speedup vs baseline: 2.0610x; 1.0092x over previous
"""Multi-head attention (B=4, S=2048, M=1024, H=16, D=64) on 8 trn2 cores.

Sharding: core c = (b, g) with b = c // 2 (batch), g = c % 2 (head group of 8
heads).  Each core computes q/k/v projections for its 8 heads, causal
attention, and a partial output projection (contraction over its 512 feature
rows of Wo).  Host sums the two partials per batch and adds the bias.

All matmul operands are fp16 (fp32 matmul trips the PE power throttle to a
0.5 util limit on trn2; fp16 streams 1 row/cycle).  PSUM accumulation stays
fp32.  Weights stay SBUF-resident.

Pipelining: the PE runs in emission order, so the per-block work is emitted
as  proj(0), attn(0){fill: proj(1)}, attn(1){fill: proj(2), finish(0)},
attn(2){fill: proj(3), finish(1)}, attn(3){fill: finish(2)}, finish(3),
where attn(j) is the score/exp/PV pipeline (Act-engine bound) and the
fillers are PE-only closures drained at head-pair boundaries to plug the
exp-gated PE gaps.  finish(j) = softmax normalization (rb broadcast matmul +
multiply) and the output projection for block j; deferring it hides the
denominator-reciprocal latency entirely.

Device-side layouts:
  xqT/xkT/xvT  [1024(m), 2048(s)]  fp16 host-transposed activations
  wq/wk/wv     [1024(m), 512(dh)]  fp16, dh = 64*h_local + d  (head-major)
  wo           [512(dh), 1024(n)]  fp16
  qT,kT        [512(dh), 2048(s)]  fp16 = (X W)^T, computed as W.T @ X.T
  v            [2048(s), 520]      fp16; per 128-row tile: cols 65h..65h+63
                                   = v_h, col 65h+64 = 1.0 (softmax denom)
  scoresT      [sk, sq]            = kT_h.T @ qT_h, head pair packed in one
                                   [128, 1024] PSUM tile (A cols 0:512, B
                                   cols 512:1024) so one exp covers both
  PT           exp(scoresT/8) fp16; causal diag masked AFTER exp by a 0/1
                                   lower-tri multiply (keeps Act off the
                                   DVE's critical path)
  outT_h       [65, sq]            = [v_h|1].T @ PT ; row 64 = sum_sk PT
  attnT        [512(dh), sq]       = outT_h / denom, heads stacked, fp16
  out_partial  [2048(s), 1024(n)]  fp16 = attnT.T @ wo   (no bias)
"""

import os
import sys

for _p in ("/opt/trn_rl_repo", "/root/.axon_site/_ro/trn_rl_repo"):
    if os.path.isdir(_p) and _p not in sys.path:
        sys.path.append(_p)

import numpy as np

B, S, M, H, D = 4, 2048, 1024, 16, 64
G = 2               # head groups (cores per batch)
HPG = H // G        # heads per group = 8
DH = HPG * D        # feature rows per group = 512
NCORES = B * G
SB = 512            # sq block (matmul N)
CK = 128            # sk chunk (matmul M / partition)
NJ = S // SB        # 4 sq blocks
NC = S // CK        # 16 sk chunks
MK = M // 128       # 8 m chunks

_PROG_CACHE = {}


def _build_program(variant):
    """variant: 'causal' | 'allones' | 'general'"""
    import concourse.bass as bass
    import concourse.bacc as bacc
    import concourse.mybir as mybir
    from concourse import tile
    from contextlib import ExitStack

    f32 = mybir.dt.float32
    f16 = mybir.dt.float16
    nc = bacc.Bacc("TRN2", target_bir_lowering=False, debug=False, num_devices=NCORES)

    xqT = nc.dram_tensor("xqT", [M, S], f16, kind="ExternalInput").ap()
    xkT = nc.dram_tensor("xkT", [M, S], f16, kind="ExternalInput").ap()
    xvT = nc.dram_tensor("xvT", [M, S], f16, kind="ExternalInput").ap()
    wq = nc.dram_tensor("wq", [M, DH], f16, kind="ExternalInput").ap()
    wk = nc.dram_tensor("wk", [M, DH], f16, kind="ExternalInput").ap()
    wv = nc.dram_tensor("wv", [M, DH], f16, kind="ExternalInput").ap()
    wo = nc.dram_tensor("wo", [DH, M], f16, kind="ExternalInput").ap()
    ind8 = nc.dram_tensor("ind8", [8, SB], f16, kind="ExternalInput").ap()
    if variant == "causal":
        tri01 = nc.dram_tensor("tri01", [128, 128], f16, kind="ExternalInput").ap()
    if variant == "general":
        maskT = nc.dram_tensor("maskT", [S, S], f32, kind="ExternalInput").ap()
    out = nc.dram_tensor("out", [S, M], f16, kind="ExternalOutput").ap()

    with tile.TileContext(nc) as tc, ExitStack() as ctx:
        ep = ctx.enter_context
        ctx.enter_context(nc.allow_low_precision(reason="fp16 matmul inputs"))
        dma = nc.sync.dma_start

        w_pool = ep(tc.tile_pool(name="w", bufs=24))
        x_pool = ep(tc.tile_pool(name="x", bufs=48))
        wo_pool = ep(tc.tile_pool(name="wo", bufs=1))
        qT_pool = ep(tc.tile_pool(name="qT", bufs=1))
        kT_pool = ep(tc.tile_pool(name="kT", bufs=1))
        v_pool = ep(tc.tile_pool(name="v", bufs=1))
        pt_pool = ep(tc.tile_pool(name="pt", bufs=4))
        at_pool = ep(tc.tile_pool(name="at", bufs=9))
        nrm_pool = ep(tc.tile_pool(name="nrm", bufs=3))
        out_pool = ep(tc.tile_pool(name="outp", bufs=4))
        misc_pool = ep(tc.tile_pool(name="misc", bufs=1))
        if variant == "general":
            mk_pool = ep(tc.tile_pool(name="mk", bufs=4))

        ps_mm = ep(tc.tile_pool(name="ps_mm", bufs=2, space="PSUM"))
        ps_sc = ep(tc.tile_pool(name="ps_sc", bufs=2, space="PSUM"))
        ps_pv = ep(tc.tile_pool(name="ps_pv", bufs=2, space="PSUM"))

        # constants
        if variant == "causal":
            tri_sb = misc_pool.tile([128, 128], f16, name="tri_sb")
            dma(tri_sb[:], tri01[:])
        ind8_sb = misc_pool.tile([8, SB], f16, name="ind8_sb")
        dma(ind8_sb[:], ind8[:])

        qT_sb = [qT_pool.tile([128, S], f16, name=f"qT{d}") for d in range(4)]
        kT_sb = [kT_pool.tile([128, S], f16, name=f"kT{d}") for d in range(4)]
        v_sb = [v_pool.tile([128, HPG * 65], f16, name=f"v{t}") for t in range(NC)]

        for t in range(NC):
            v3 = v_sb[t].rearrange("p (h c) -> p h c", h=HPG, c=65)
            nc.gpsimd.memset(v3[:, :, 64:65], 1.0)

        # resident weights: loaded once up front (gpsimd queue, idle at start)
        w_sb = {}
        for w_dram, kind in ((wq, "q"), (wk, "k"), (wv, "v")):
            ch = []
            for mc in range(MK):
                wt = w_pool.tile([128, DH], f16, name=f"w_{kind}{mc}", tag="w")
                nc.gpsimd.dma_start(wt[:], w_dram[mc * 128:(mc + 1) * 128, :])
                ch.append(wt)
            w_sb[kind] = ch
        wo_sb = []
        for d in range(4):
            wt = wo_pool.tile([128, M], f16, name=f"wo{d}")
            nc.gpsimd.dma_start(wt[:], wo[d * 128:(d + 1) * 128, :])
            wo_sb.append(wt)

        _x_dram = {"q": xqT, "k": xkT, "v": xvT}

        def prefetch_x(j, kinds="qkv"):
            """Issue the x DMAs for s-block j now (sync queue runs well ahead
            of the PE, so the data lands before the proj groups need it)."""
            xmap = {}
            for kind in kinds:
                x_ch = []
                for mc in range(MK):
                    xt = x_pool.tile(
                        [128, SB], f16, name=f"x_{kind}{j}_{mc}", tag="x")
                    dma(xt[:], _x_dram[kind][mc * 128:(mc + 1) * 128,
                                             j * SB:(j + 1) * SB])
                    x_ch.append(xt)
                xmap[kind] = x_ch
            return xmap

        def proj_closures(j, xmap, kinds="qkv"):
            """4 PE-group closures per kind computing qT/kT/v for s-block j."""
            closures = []
            for kind in kinds:
                x_ch = xmap[kind]
                if kind in ("q", "k"):
                    dst = qT_sb if kind == "q" else kT_sb

                    def group(d, kind=kind, x_ch=x_ch, dst=dst):
                        ps = ps_mm.tile(
                            [128, SB], f32, name=f"ps_{kind}{j}_{d}", tag="mm")
                        for mc in range(MK):
                            nc.tensor.matmul(
                                ps[:], w_sb[kind][mc][:, d * 128:(d + 1) * 128],
                                x_ch[mc][:],
                                start=(mc == 0), stop=(mc == MK - 1))
                        nc.vector.tensor_copy(
                            dst[d][:, j * SB:(j + 1) * SB], ps[:])
                else:
                    def group(st, x_ch=x_ch):
                        t = 4 * j + st
                        ps = ps_mm.tile([128, DH], f32, name=f"ps_v{t}", tag="mm")
                        for mc in range(MK):
                            nc.tensor.matmul(
                                ps[:], x_ch[mc][:, st * 128:(st + 1) * 128],
                                w_sb["v"][mc][:],
                                start=(mc == 0), stop=(mc == MK - 1))
                        v3 = v_sb[t].rearrange("p (h c) -> p h c", h=HPG, c=65)
                        p3 = ps.rearrange("p (h c) -> p h c", h=HPG, c=64)
                        nc.vector.tensor_copy(v3[:, :, 0:64], p3[:])

                for d in range(4):
                    closures.append(lambda d=d, group=group: group(d))
            return closures

        def emit_attn(j, fillers, early_fillers=()):
            """Scores / exp / PV for s-block j; drains filler closures at
            head-pair boundaries (and mid-chunk-loop for the big blocks).
            early_fillers are drained one per chunk at the start of hp0 —
            used for proj(j) k/v groups that later chunks of this very
            block depend on.  Returns the state needed by finish_closures."""
            nchunks = 4 * (j + 1) if variant == "causal" else NC
            eq = list(early_fillers)
            fq = list(fillers)
            nhp = HPG // 2
            # distribute fillers over (hp, drain-site) slots
            drains_per_hp = max(1, (len(fq) + nhp - 1) // nhp) if fq else 0

            def drain(n):
                for _ in range(n):
                    if eq:
                        eq.pop(0)()
                    elif fq:
                        fq.pop(0)()

            at_tiles = []
            dn_all = nrm_pool.tile([8, SB], f32, name=f"dn{j}", tag="dn")
            for hp in range(nhp):
                hA, hB = 2 * hp, 2 * hp + 1
                dtile = hp  # == hA//2 == hB//2
                at = at_pool.tile([128, SB], f16, name=f"at{j}_{hp}", tag="at")
                at_tiles.append(at)
                pvA = ps_pv.tile([65, SB], f32, name=f"pv{j}_{hA}", tag="pv")
                pvB = ps_pv.tile([65, SB], f32, name=f"pv{j}_{hB}", tag="pv")
                for c in range(nchunks):
                    o = 0
                    diag = variant == "causal" and c >= 4 * j
                    if diag:
                        o = 128 * (c - 4 * j)
                    # paired scores: head A cols [o:SB], head B [SB+o:2*SB]
                    sc = ps_sc.tile([128, 2 * SB], f32,
                                    name=f"sc{j}_{hp}_{c}", tag="sc")
                    nc.tensor.matmul(
                        sc[:, o:SB],
                        kT_sb[dtile][0:64, c * CK:(c + 1) * CK],
                        qT_sb[dtile][0:64, j * SB + o:(j + 1) * SB],
                        start=True, stop=True)
                    nc.tensor.matmul(
                        sc[:, SB + o:2 * SB],
                        kT_sb[dtile][64:128, c * CK:(c + 1) * CK],
                        qT_sb[dtile][64:128, j * SB + o:(j + 1) * SB],
                        start=True, stop=True)
                    pt = pt_pool.tile([128, 2 * SB], f16,
                                      name=f"pt{j}_{hp}_{c}", tag="pt")
                    if variant == "general":
                        mk = mk_pool.tile([128, SB], f32,
                                          name=f"mk{j}_{hp}_{c}", tag="mk")
                        nc.gpsimd.dma_start(
                            mk[:], maskT[c * CK:(c + 1) * CK,
                                         j * SB:(j + 1) * SB])
                        nc.vector.tensor_add(sc[:, 0:SB], sc[:, 0:SB], mk[:])
                        nc.vector.tensor_add(
                            sc[:, SB:2 * SB], sc[:, SB:2 * SB], mk[:])
                    if diag:
                        nc.scalar.activation(
                            pt[:, o:SB], sc[:, o:SB],
                            mybir.ActivationFunctionType.Exp, scale=0.125)
                        nc.scalar.activation(
                            pt[:, SB + o:2 * SB], sc[:, SB + o:2 * SB],
                            mybir.ActivationFunctionType.Exp, scale=0.125)
                        # mask after exp: multiply diag 128-col block by 0/1 tri
                        nc.vector.tensor_mul(
                            pt[:, o:o + 128], pt[:, o:o + 128], tri_sb[:])
                        nc.vector.tensor_mul(
                            pt[:, SB + o:SB + o + 128],
                            pt[:, SB + o:SB + o + 128], tri_sb[:])
                    else:
                        nc.scalar.activation(
                            pt[:, 0:2 * SB], sc[:, 0:2 * SB],
                            mybir.ActivationFunctionType.Exp, scale=0.125)
                    nc.tensor.matmul(
                        pvA[:, o:SB], v_sb[c][:, 65 * hA:65 * hA + 65],
                        pt[:, o:SB],
                        start=(c == 0), stop=(c == nchunks - 1))
                    nc.tensor.matmul(
                        pvB[:, o:SB], v_sb[c][:, 65 * hB:65 * hB + 65],
                        pt[:, SB + o:2 * SB],
                        start=(c == 0), stop=(c == nchunks - 1))
                    # early fillers: drip one per chunk so proj(j) k/v
                    # complete before the chunks that read block j
                    if eq and hp == 0:
                        eq.pop(0)()
                    # mid-loop drip for the long chunk loops
                    if nchunks >= 12 and c in (5, 10):
                        drain(1)
                for h, pv in ((hA, pvA), (hB, pvB)):
                    drow = 64 * (h % 2)
                    nc.vector.tensor_copy(at[drow:drow + 64, :], pv[0:64, :])
                    # DVE writes must start at a quarter partition; bounce the
                    # denominator row through partition 0, DMA to row h
                    dnt = nrm_pool.tile([1, SB], f32, name=f"dnt{j}_{h}",
                                        tag="dnt")
                    nc.vector.tensor_copy(dnt[:], pv[64:65, :])
                    nc.gpsimd.dma_start(dn_all[h:h + 1, :], dnt[:])
                drain(drains_per_hp)
            drain(len(fq))
            # approx reciprocal (~5x cheaper than nc.vector.reciprocal; the
            # denominators are O(1e2..1e3), far from its undefined edges)
            dnr32 = nrm_pool.tile([8, SB], f32, name=f"dnr32{j}", tag="dnr32")
            nc.vector.reciprocal_approx_fast(dnr32[:], dn_all[:])
            dnr = nrm_pool.tile([8, SB], f16, name=f"dnr{j}", tag="dnr")
            nc.vector.tensor_copy(dnr[:], dnr32[:])
            return at_tiles, dnr

        def finish_closures(j, at_tiles, dnr):
            """12 PE closures: normalize attnT and run the output projection."""
            closures = []

            def rb_mul(hp):
                rb = ps_mm.tile([128, SB], f32, name=f"rb{j}_{hp}", tag="mm")
                nc.tensor.matmul(
                    rb[:], ind8_sb[:, hp * 128:(hp + 1) * 128], dnr[:],
                    start=True, stop=True)
                nc.vector.tensor_mul(at_tiles[hp][:], at_tiles[hp][:], rb[:])

            def outproj(ss, nh):
                ps = ps_mm.tile([128, SB], f32, name=f"po{j}_{ss}_{nh}", tag="mm")
                for d in range(4):
                    nc.tensor.matmul(
                        ps[:],
                        at_tiles[d][:, ss * 128:(ss + 1) * 128],
                        wo_sb[d][:, nh * SB:(nh + 1) * SB],
                        start=(d == 0), stop=(d == 3))
                ot = out_pool.tile([128, SB], f16, name=f"ot{j}_{ss}_{nh}",
                                   tag="ot")
                nc.vector.tensor_copy(ot[:], ps[:])
                r0 = j * SB + ss * 128
                # alternate store queues so the final drain isn't serialized
                q = nc.sync if (ss + nh) % 2 else nc.gpsimd
                q.dma_start(out[r0:r0 + 128, nh * SB:(nh + 1) * SB], ot[:])

            for hp in range(HPG // 2):
                closures.append(lambda hp=hp: rb_mul(hp))
            for ss in range(4):
                for nh in range(2):
                    closures.append(lambda ss=ss, nh=nh: outproj(ss, nh))
            return closures

        if variant == "causal":
            # Filler schedule: attn(3) is the Act-heaviest block and has no
            # proj(4) to hide behind, so proj(3)'s k/v groups are pushed into
            # attn(3) itself as early fillers (they complete before chunk 12,
            # the first chunk that reads block 3).
            #   proj(0); attn(0){proj(1)}; attn(1){proj(2), finish(0)};
            #   attn(2){proj(3).q, finish(1)}; attn(3){early: proj(3).kv,
            #   finish(2)}; finish(3)
            for cl in proj_closures(0, prefetch_x(0)):
                cl()
            x1 = prefetch_x(1)
            at0, dnr0 = emit_attn(0, proj_closures(1, x1))
            fin0 = finish_closures(0, at0, dnr0)
            x2 = prefetch_x(2)
            at1, dnr1 = emit_attn(1, proj_closures(2, x2) + fin0)
            fin1 = finish_closures(1, at1, dnr1)
            x3 = prefetch_x(3)
            at2, dnr2 = emit_attn(2, proj_closures(3, x3, "q") + fin1)
            fin2 = finish_closures(2, at2, dnr2)
            at3, dnr3 = emit_attn(
                3, fin2, early_fillers=proj_closures(3, x3, "kv"))
            for cl in finish_closures(3, at3, dnr3):
                cl()
        else:
            # non-causal: every attn(j) reads all k/v blocks, so run all
            # projections first, then the attention blocks
            for j in range(NJ):
                for cl in proj_closures(j, prefetch_x(j)):
                    cl()
            pending = []
            for j in range(NJ):
                at_j, dnr_j = emit_attn(j, pending)
                pending = finish_closures(j, at_j, dnr_j)
            for cl in pending:
                cl()

    nc.compile()
    return nc


def _get_program(variant):
    if variant not in _PROG_CACHE:
        _PROG_CACHE[variant] = _build_program(variant)
    return _PROG_CACHE[variant]


def _host_prep(queries, keys, values, masks, Wq, Wk, Wv):
    """Build the 8 per-core input maps."""
    tril = np.tril(np.ones((S, S), dtype=bool))
    if all(np.array_equal(masks[b], tril) for b in range(B)):
        variant = "causal"
    elif masks.all():
        variant = "allones"
    else:
        variant = "general"

    sq = np.arange(128)
    tri01_np = (sq[None, :] >= sq[:, None]).astype(np.float16)
    ind8_np = np.zeros((8, 512), np.float16)
    for c in range(4):
        for cc in range(128):
            ind8_np[2 * c + cc // 64, 128 * c + cc] = 1.0

    # [H, M, D] -> [M, H*D] head-major per group
    def wcat(w, g):
        return np.ascontiguousarray(
            w[g * HPG:(g + 1) * HPG].transpose(1, 0, 2).reshape(M, DH)
        ).astype(np.float16)

    in_maps = []
    for c in range(NCORES):
        b, g = c // G, c % G
        m = {
            "xqT": np.ascontiguousarray(queries[b].T).astype(np.float16),
            "xkT": np.ascontiguousarray(keys[b].T).astype(np.float16),
            "xvT": np.ascontiguousarray(values[b].T).astype(np.float16),
            "wq": wcat(Wq, g),
            "wk": wcat(Wk, g),
            "wv": wcat(Wv, g),
            "ind8": ind8_np,
        }
        if variant == "causal":
            m["tri01"] = tri01_np
        if variant == "general":
            m["maskT"] = np.where(masks[b].T, 0.0, -1.0e6).astype(np.float32)
        in_maps.append(m)
    return variant, in_maps


def run(queries, keys, values, masks, Wq, Wk, Wv, Wo, bo, trace=False):
    from concourse import bass_utils

    queries = np.asarray(queries, np.float32)
    keys = np.asarray(keys, np.float32)
    values = np.asarray(values, np.float32)
    masks = np.asarray(masks, bool)
    Wq = np.asarray(Wq, np.float32)
    Wk = np.asarray(Wk, np.float32)
    Wv = np.asarray(Wv, np.float32)
    Wo = np.asarray(Wo, np.float32)
    bo = np.asarray(bo, np.float32)

    variant, in_maps = _host_prep(queries, keys, values, masks, Wq, Wk, Wv)
    for c in range(NCORES):
        g = c % G
        in_maps[c]["wo"] = np.ascontiguousarray(
            Wo[g * DH:(g + 1) * DH, :]).astype(np.float16)

    nc = _get_program(variant)
    res = bass_utils.run_bass_kernel_spmd(
        nc, in_maps, list(range(NCORES)), trace=trace)

    out = np.empty((B, S, M), np.float32)
    for b in range(B):
        out[b] = (res.results[G * b]["out"].astype(np.float32)
                  + res.results[G * b + 1]["out"].astype(np.float32) + bo)
    return out, res


def kernel(queries, keys, values, masks, Wq, Wk, Wv, Wo, bo):
    out, _ = run(queries, keys, values, masks, Wq, Wk, Wv, Wo, bo, trace=False)
    return out
